# revision 1
# baseline (speedup 1.0000x reference)
"""HausdorffDT loss kernel for Trainium2 (8 NeuronCores, Bass/Tile).

Math: with ALPHA=2 and field(m) = sqrt(edt2(m)) + sqrt(edt2(~m)), one of the
two terms is zero at every pixel, so field(m)^2 == edt2(m) + edt2(~m) exactly.
The loss is therefore

    mean( (x - onehot)^2 * (edt2(pm)+edt2(~pm) + edt2(tm)+edt2(~tm)) )

with an all-zero-field guard per empty mask.  Squared EDTs are exact small
integers, so the whole distance pipeline runs in bf16 exactly:

  1. row pass: exact 1D distance to nearest True along W via two
     tensor_tensor_scan min-plus recurrences (fwd + bwd), batched over all
     fields with INF padding between row segments (leaked state across a pad
     is >= PAD >= clamp value, hence harmless after clamping).
  2. clamp at Vc = R+1 (host-verified R bounds the true max distance).
  3. DMA-xbar transpose of the clamped row distances (2-byte dtype).
  4. column pass: windowed parabola min-plus
     acc = min(acc, g[i +- d] + d^2), d = 1..R, exact because the optimal
     vertical offset is bounded by the true distance <= R.
  5. transpose back, weighted reduce against (x - onehot)^2 in fp32,
     per-(class, kind) partial sums; host applies empty-mask guards + mean.

Sharding: data-parallel over batch, one sample per core; partial sums are
combined on the host (no collectives needed for a scalar loss).

Host-side metadata (window radius R per mask kind, guards) is recomputed from
the actual inputs on every call; if the inputs ever violate the window bound
(R > 15) or contain an all-True mask, a slow exact numpy fallback is used.
"""

import numpy as np

B, C, H, W = 8, 4, 256, 256
NCORES = 8
P = 128
PAD = 16
SEG = W + 2 * PAD          # 288 columns per row segment
NSEG = 32                  # (kind 2) x (pol 2) x (class 4) x (chunk 2)
FREE_A = NSEG * SEG        # 9216
INF = 4096.0               # "no pixel" marker for the scans
PADV = 64.0                # pad value in transposed tiles; squared -> 4096
BIG = float(H + W)
R_CAP = 15                 # pads support windows up to 15 (Vc = R+1 <= PAD)

_CACHE = {}
LAST_RESULT = None  # BassKernelResults of the most recent run (for profiling)
LAST_EXEC_WALL_NS = None  # wall-clock of run_bass_kernel_spmd (compile+run)


# ----------------------------------------------------------------- host side

def _seg(k, t, c, h):
    return k * 16 + t * 8 + c * 2 + h


def _annulus_offsets():
    """Offsets grouped by squared radius, up to R_CAP."""
    by_r2 = {}
    for di in range(-R_CAP, R_CAP + 1):
        for dj in range(-R_CAP, R_CAP + 1):
            r2 = di * di + dj * dj
            if 0 < r2 <= R_CAP * R_CAP:
                by_r2.setdefault(r2, []).append((di, dj))
    return sorted(by_r2.items())


def _shift_or(dst, src, di, dj):
    """dst |= shift(src, di, dj) with zero fill; arrays [N,H,W]."""
    hs = slice(max(di, 0), H + min(di, 0))
    hd = slice(max(-di, 0), H + min(-di, 0))
    ws = slice(max(dj, 0), W + min(dj, 0))
    wd = slice(max(-dj, 0), W + min(-dj, 0))
    dst[:, hd, wd] |= src[:, hs, ws]


def _required_R(masks):
    """masks: [N,H,W] bool, each with both colors present.  Returns minimal
    integer R such that every pixel has an opposite-color pixel within
    Euclidean distance R, or None if that exceeds R_CAP."""
    if masks.shape[0] == 0:
        return 1
    covT = masks.copy()       # dilation of True set
    covF = ~masks             # dilation of False set
    def done():
        cov = np.where(masks, covF, covT)
        return cov.all()
    if done():
        return 1  # R>=1 minimum window
    for r2, offs in _annulus_offsets():
        for (di, dj) in offs:
            _shift_or(covT, masks, di, dj)
            _shift_or(covF, ~masks, di, dj)
        if done():
            # window only needs |di| <= floor(d_max); Vc = R+1 > d_max holds
            return max(1, int(np.floor(np.sqrt(r2) + 1e-9)))
    return None


def _loss_numpy_exact(x, y):
    """Slow exact replica of the reference (float32 math, float64 mean)."""
    def dist1d(z):
        n = z.shape[-1]
        idx = np.arange(n, dtype=np.int64)
        fw = np.where(z, idx, -1)
        fw = np.maximum.accumulate(fw, axis=-1)
        df = np.where(fw >= 0, (idx - fw).astype(np.float32), np.float32(BIG))
        bw = np.where(z, idx, 2 * n)[..., ::-1]
        bw = np.minimum.accumulate(bw, axis=-1)[..., ::-1]
        db = np.where(bw < 2 * n, (bw - idx).astype(np.float32), np.float32(BIG))
        return np.minimum(df, db)

    def edt_sq(z):  # [H,W] bool -> squared EDT to True set
        g = dist1d(z).astype(np.float32) ** 2
        i = np.arange(H, dtype=np.float32)
        out = np.empty((H, W), np.float32)
        for i0 in range(0, H, 32):
            off = (i[i0:i0 + 32, None] - i[None, :]) ** 2      # [32,H]
            out[i0:i0 + 32] = (off[:, :, None] + g[None, :, :]).min(axis=1)
        return out

    def field(m):
        if not m.any():
            return np.zeros((H, W), np.float32)
        return np.sqrt(edt_sq(~m)) + np.sqrt(edt_sq(m))

    total = 0.0
    for b in range(B):
        for c in range(C):
            oh = (y[b] == c)
            pm = x[b, c] > 0.5
            dist = field(pm).astype(np.float32) ** 2 + field(oh).astype(np.float32) ** 2
            w = (x[b, c] - oh.astype(np.float32)) ** 2
            total += float((w.astype(np.float64) * dist.astype(np.float64)).sum())
    return np.float32(total / (B * C * H * W))


# --------------------------------------------------------------- bass kernel

def _build(R_pred, R_tgt):
    import concourse.bacc as bacc
    import concourse.mybir as mybir
    from concourse.tile import TileContext

    dt = mybir.dt
    op = mybir.AluOpType
    Vc = {0: float(R_pred + 1), 1: float(R_tgt + 1)}
    Rk = {0: R_pred, 1: R_tgt}

    nc = bacc.Bacc("TRN2", target_bir_lowering=False, debug=False,
                   enable_asserts=False, num_devices=NCORES)
    xb = nc.dram_tensor("x", [C, H, W], dt.float32, kind="ExternalInput")
    yb = nc.dram_tensor("y", [H, W], dt.int32, kind="ExternalInput")
    ob = nc.dram_tensor("out", [1, 8], dt.float32, kind="ExternalOutput")

    with TileContext(nc) as tc:
        with tc.tile_pool(name="main", bufs=1) as pool:
            x_sb = pool.tile([P, C * 2 * W], dt.float32, tag="x_sb")
            y_sb = pool.tile([P, 2 * W], dt.int32, tag="y_sb")
            m_pred = pool.tile([P, C * 2 * W], dt.bfloat16, tag="m_pred")
            m_tgt = pool.tile([P, C * 2 * W], dt.bfloat16, tag="m_tgt")
            a = pool.tile([P, FREE_A], dt.bfloat16, tag="a")
            ones = pool.tile([P, FREE_A], dt.bfloat16, tag="ones")
            f = pool.tile([P, FREE_A], dt.bfloat16, tag="f")
            d1T = pool.tile([P, FREE_A], dt.bfloat16, tag="d1T")
            gT = pool.tile([P, FREE_A], dt.bfloat16, tag="gT")
            acc = pool.tile([P, FREE_A], dt.bfloat16, tag="acc")
            Sn = pool.tile([P, 2 * C * 2 * W], dt.bfloat16, tag="Sn")
            tdiff = pool.tile([P, C * 2 * W], dt.float32, tag="tdiff")
            wsq = pool.tile([P, C * 2 * W], dt.float32, tag="wsq")
            junk = [pool.tile([P, 2 * W], dt.float32, tag=f"junk{i}",
                              name=f"junk{i}") for i in range(8)]
            cols = pool.tile([P, 8], dt.float32, tag="cols")
            fin = pool.tile([P, 8], dt.float32, tag="fin")

            def segview(tile, s0, n, lo, hi):
                v = tile[:, s0 * SEG:(s0 + n) * SEG]
                v = v.rearrange("p (s w) -> p s w", w=SEG)
                return v[:, :, lo:hi]

            # ---- loads
            nc.sync.dma_start(
                out=x_sb[:, :].rearrange("p (c hh w) -> p c hh w", c=C, hh=2),
                in_=xb.ap().rearrange("c (hh p) w -> p c hh w", p=P))
            nc.sync.dma_start(
                out=y_sb[:, :].rearrange("p (hh w) -> p hh w", hh=2),
                in_=yb.ap().rearrange("(hh p) w -> p hh w", p=P))

            # ---- masks (bf16 0/1)
            nc.vector.tensor_scalar(out=m_pred[:, :], in0=x_sb[:, :],
                                    scalar1=0.5, scalar2=None, op0=op.is_gt)
            for c in range(C):
                nc.vector.tensor_scalar(
                    out=m_tgt[:, c * 2 * W:(c + 1) * 2 * W],
                    in0=y_sb[:, :], scalar1=float(c), scalar2=None,
                    op0=op.is_equal)

            # ---- scan input a: 0 where zero-set pixel, INF elsewhere
            nc.vector.memset(segview(a, 0, NSEG, 0, PAD), INF)
            nc.vector.memset(segview(a, 0, NSEG, SEG - PAD, SEG), INF)
            for k, m in ((0, m_pred), (1, m_tgt)):
                mv = m[:, :].rearrange("p (s w) -> p s w", w=W)
                # pol T: dist to True pixels  -> a = INF*(1-m)
                nc.vector.tensor_scalar(
                    out=segview(a, k * 16, 8, PAD, PAD + W), in0=mv,
                    scalar1=-INF, scalar2=INF, op0=op.mult, op1=op.add)
                # pol F: dist to False pixels -> a = INF*m
                nc.vector.tensor_scalar(
                    out=segview(a, k * 16 + 8, 8, PAD, PAD + W), in0=mv,
                    scalar1=INF, scalar2=None, op0=op.mult)

            # ---- row pass: d1[j] = min_j' |j-j'| s.t. zero-set, via 2 scans
            nc.vector.memset(ones[:, :], 1.0)
            nc.vector.tensor_tensor_scan(
                out=f[:, :], data0=ones[:, :], data1=a[:, :],
                initial=INF, op0=op.add, op1=op.min)
            nc.vector.tensor_tensor_scan(
                out=a[:, ::-1], data0=ones[:, ::-1], data1=f[:, ::-1],
                initial=INF, op0=op.add, op1=op.min)
            # a now holds d1; clamp per kind at Vc (> true max distance)
            for k in range(2):
                nc.vector.tensor_scalar(
                    out=a[:, k * 16 * SEG:(k + 1) * 16 * SEG],
                    in0=a[:, k * 16 * SEG:(k + 1) * 16 * SEG],
                    scalar1=Vc[k], scalar2=None, op0=op.min)

            # ---- transpose d1 into d1T ([W-half, H] layout)
            nc.vector.memset(segview(d1T, 0, NSEG, 0, PAD), PADV)
            nc.vector.memset(segview(d1T, 0, NSEG, SEG - PAD, SEG), PADV)
            dma_engines = (nc.sync, nc.scalar)
            n_t = 0
            for k in range(2):
                for t in range(2):
                    for c in range(C):
                        for h in range(2):
                            for v in range(2):
                                src = a[:, _seg(k, t, c, h) * SEG + PAD + 128 * v:
                                        _seg(k, t, c, h) * SEG + PAD + 128 * (v + 1)]
                                dst = d1T[:, _seg(k, t, c, v) * SEG + PAD + 128 * h:
                                          _seg(k, t, c, v) * SEG + PAD + 128 * (h + 1)]
                                dma_engines[n_t % 2].dma_start_transpose(out=dst, in_=src)
                                n_t += 1

            # ---- g = d1^2 (pads -> 4096)
            nc.scalar.square(out=gT[:, :], in_=d1T[:, :])

            # ---- column pass: acc = min_d ( g[i+-d] + d^2 ), d = 0..Rk
            dmax = max(R_pred, R_tgt)
            first = True
            for d in range(1, dmax + 1):
                ks = [k for k in range(2) if Rk[k] >= d]
                s0 = 0 if ks[0] == 0 else 16
                n = 16 * len(ks)
                assert ks == list(range(ks[0], ks[0] + len(ks)))
                for sgn in (+1, -1):
                    in0 = segview(gT, s0, n, PAD + sgn * d, PAD + sgn * d + W)
                    in1 = segview(gT if first else acc, s0, n, PAD, PAD + W)
                    nc.vector.scalar_tensor_tensor(
                        out=segview(acc, s0, n, PAD, PAD + W),
                        in0=in0, scalar=float(d * d), in1=in1,
                        op0=op.add, op1=op.min)
                    first = False

            # ---- S = edt2(m) + edt2(~m): accT += accF (in place, T half)
            for k in range(2):
                nc.vector.tensor_add(
                    out=segview(acc, k * 16, 8, PAD, PAD + W),
                    in0=segview(acc, k * 16, 8, PAD, PAD + W),
                    in1=segview(acc, k * 16 + 8, 8, PAD, PAD + W))

            # ---- transpose S back to row-major Sn
            n_t = 0
            for k in range(2):
                for c in range(C):
                    for h in range(2):
                        for v in range(2):
                            src = acc[:, _seg(k, 0, c, v) * SEG + PAD + 128 * h:
                                      _seg(k, 0, c, v) * SEG + PAD + 128 * (h + 1)]
                            base = ((k * C + c) * 2 + h) * W
                            dst = Sn[:, base + 128 * v: base + 128 * (v + 1)]
                            dma_engines[n_t % 2].dma_start_transpose(out=dst, in_=src)
                            n_t += 1

            # ---- weighted partial sums: sum((x-onehot)^2 * S) per (kind,class)
            nc.vector.tensor_sub(out=tdiff[:, :], in0=x_sb[:, :], in1=m_tgt[:, :])
            nc.scalar.square(out=wsq[:, :], in_=tdiff[:, :])
            for k in range(2):
                for c in range(C):
                    i = k * C + c
                    nc.vector.tensor_tensor_reduce(
                        out=junk[i][:, :],
                        in0=wsq[:, c * 2 * W:(c + 1) * 2 * W],
                        in1=Sn[:, i * 2 * W:(i + 1) * 2 * W],
                        scale=1.0, scalar=0.0,
                        op0=op.mult, op1=op.add,
                        accum_out=cols[:, i:i + 1])
            nc.gpsimd.tensor_reduce(out=fin[0:1, 0:8], in_=cols[:, 0:8],
                                    axis=mybir.AxisListType.C, op=op.add)
            nc.sync.dma_start(out=ob.ap(), in_=fin[0:1, 0:8])

    nc.compile()
    return nc


def _ensure_ntff_hook_shim():
    """This image's antenv lacks axon_hooks; provide it so trace=True works."""
    import sys, types
    if "antenv.axon_hooks" in sys.modules:
        return
    mod = types.ModuleType("antenv.axon_hooks")
    _hook = [None]
    def set_axon_ntff_profile_hook(h):
        _hook[0] = h
    def get_axon_ntff_profile_hook():
        if _hook[0] is None:
            try:
                from trn_agent_boot.trn_boot import _ntff_profile_via_ctypes
                _hook[0] = _ntff_profile_via_ctypes("/opt/axon/libaxon_pjrt.so")
            except Exception:
                return None
        return _hook[0]
    mod.set_axon_ntff_profile_hook = set_axon_ntff_profile_hook
    mod.get_axon_ntff_profile_hook = get_axon_ntff_profile_hook
    sys.modules["antenv.axon_hooks"] = mod


# ------------------------------------------------------------------- driver

def kernel(x, y):
    x = np.ascontiguousarray(np.asarray(x, np.float32))
    y = np.ascontiguousarray(np.asarray(y, np.int32))
    assert x.shape == (B, C, H, W) and y.shape == (B, H, W)

    pred = x > 0.5
    oh = np.stack([y == c for c in range(C)], axis=1)          # [B,C,H,W]
    g_pred = pred.reshape(B * C, -1).any(axis=1)
    g_tgt = oh.reshape(B * C, -1).any(axis=1)

    # masks that matter must have both colors present and bounded distances
    def check_kind(masks, guards):
        live = masks.reshape(B * C, H, W)[guards]
        if live.shape[0] and not (~live.reshape(live.shape[0], -1)).any(axis=1).all():
            return None  # some all-True mask -> unbounded field
        return _required_R(live)

    R_pred = check_kind(pred, g_pred)
    R_tgt = check_kind(oh, g_tgt)
    if R_pred is None or R_tgt is None:
        return _loss_numpy_exact(x, y)

    try:
        _ensure_ntff_hook_shim()
        from concourse.bass_utils import run_bass_kernel_spmd

        key = (R_pred, R_tgt)
        if key not in _CACHE:
            _CACHE[key] = _build(R_pred, R_tgt)
        nc = _CACHE[key]

        import time
        in_maps = [{"x": x[b], "y": y[b]} for b in range(B)]
        t0 = time.perf_counter()
        res = run_bass_kernel_spmd(nc, in_maps, core_ids=list(range(NCORES)))
        global LAST_RESULT, LAST_EXEC_WALL_NS
        LAST_RESULT = res
        LAST_EXEC_WALL_NS = int((time.perf_counter() - t0) * 1e9)
    except Exception as e:  # device unavailable etc. -> exact host fallback
        import sys
        print(f"kernel: device path failed ({type(e).__name__}: {e}); "
              "using exact host fallback", file=sys.stderr)
        return _loss_numpy_exact(x, y)
    partials = np.stack([res.results[b]["out"].reshape(2, C) for b in range(B)])
    guards = np.stack([g_pred.reshape(B, C), g_tgt.reshape(B, C)], axis=1)
    total = float((partials.astype(np.float64) * guards).sum())
    return np.asarray(np.float32(total / (B * C * H * W)))

